# revision 19
# baseline (speedup 1.0000x reference)
"""9x9 morphological dilation (sliding-window max, SAME padding) on Trainium2.

Input : label (16, 1024, 1024, 1) float32, values in [0, 1).
Output: same shape; out[b,i,j] = max over the 9x9 window centered at (i,j),
        clipped to the image (cv2-style border handling for dilate).

Strategy (per NeuronCore; batch is data-parallel over 8 cores, 2 images/core).
Measured on this part, engine/DMA phases barely overlap, so the design
minimizes TOTAL work and instruction count rather than balancing engines:
a minimal load -> DVE-only compute -> store pipeline (~13 instructions per
chunk), with the f32->fp16 conversion fused into the first tree level.

  - SBUF layout: 128 partitions = 2 images x 64 row-blocks.  Partition q
    holds image rows 16q-4 .. 16q+19 (24 rows: the 16-row block shifted by
    the window radius, plus an 8-row halo loaded directly from HBM), so
    R9[q,r] = max over rows (16q+r-4)..(16q+r+4) = output row 16q+r: output
    rows align exactly with partitions and each image chunk stores with ONE
    contiguous DMA.  Pad rows at the image top/bottom stay zero via a
    one-time memset of the persistent X tiles (inputs are >= 0, so 0 is a
    valid -inf substitute).
  - Vertical 9-max: fully local DVE log tree: T2 (f32 inputs, fp16 output --
    the dtype conversion rides the first pass), T4 (+2 rows), T8 (+4 rows),
    R9[r] = max(T8[r], T2[r+7]) covering rows r..r+8.
  - Horizontal 9-max: DVE log tree along the free dim (+1,+2,+4 col shifts,
    then a merge with R9[c+8] writing f32).  fp16 ops run at the DVE 2x
    rate (~0.5 ns/elem measured); T2 and the merge pay the f32 rate once
    each.
  - Chunks [128, 232, 232, 232, 200]: narrow first chunk shortens the
    startup chain, narrow last shortens the drain.  Loads are prefetched
    two chunks ahead into 3 persistent X tiles and split across the ACT/SP
    DMA queues; stores ride the opposite queue per image.
"""

import numpy as np

B, H, W = 16, 1024, 1024
NCORES = 8
IMGS = 2            # images per core
RB = 16             # rows per partition block
HALO = 8            # vertical halo rows (window 9 -> 8)
SH = 4              # row shift (window radius)
XR = RB + HALO      # rows held per partition (24)
CHUNKS = [128, 232, 232, 232, 200]   # output cols per chunk (sum = 1024)
assert sum(CHUNKS) == W

_CACHE = {}


def _build(reps=1):
    import concourse.bacc as bacc
    import concourse.tile as tile
    import concourse.mybir as mybir

    f32 = mybir.dt.float32
    f16 = mybir.dt.float16

    nc = bacc.Bacc("TRN2", target_bir_lowering=False, debug=False, num_devices=1)
    x = nc.dram_tensor("x", [IMGS, H, W], f32, kind="ExternalInput").ap()
    y = nc.dram_tensor("y", [IMGS, H, W], f32, kind="ExternalOutput").ap()

    # shifted view: entry q (0..62) covers image rows 16q+12 .. 16q+27;
    # used both for the 16-row main blocks (-> partitions q+1) and, via its
    # first 8 rows, for the halo rows of partitions 0..62.  The leading i
    # dim lets one DMA cover both images (partition AP [2, 63]).
    xv4 = x[:, RB - SH:H - SH, :].rearrange("i (q r) c -> i q r c", r=RB)
    xtop = x[:, 0:RB - SH, :].rearrange("i (q r) c -> i q r c", q=1)
    xbot = x[:, H - SH:H, :].rearrange("i (q r) c -> i q r c", q=1)
    yv4 = y.rearrange("i (q r) c -> i q r c", r=RB)

    nchunk = len(CHUNKS)
    chunk_off = np.cumsum([0] + CHUNKS[:-1]).tolist()
    U = [cw + 2 * SH for cw in CHUNKS]
    UMAX = max(U)

    with tile.TileContext(nc) as tc:
        with (
            tc.tile_pool(name="px", bufs=1) as px,
            tc.tile_pool(name="pt2", bufs=1) as pt2,
            tc.tile_pool(name="pt4", bufs=1) as pt4,
            tc.tile_pool(name="pt8", bufs=1) as pt8,
            tc.tile_pool(name="pr9", bufs=2) as pr9,
            tc.tile_pool(name="pg", bufs=2) as pg,
            tc.tile_pool(name="pout", bufs=1) as pout,
        ):
            # three persistent f32 input tiles (prefetch depth 2).  One-time
            # full zeroing keeps the image top/bottom pad rows (q=0 tile
            # rows 0..3, q=63 tile rows 20..23) valid forever; per-chunk
            # loads only overwrite the real-data regions.
            x_tiles = []
            for s in range(3):
                t = px.tile([128, XR * UMAX], f32, tag=f"x{s}")
                t3 = t.rearrange("p (r u) -> p r u", u=UMAX)
                t4 = t.rearrange("(i p) (r u) -> i p r u", i=IMGS, u=UMAX)
                nc.gpsimd.memset(t[:], 0.0)
                x_tiles.append((t3, t4))

            def emit_load(it):
                ch = it % nchunk
                cw = CHUNKS[ch]
                u = U[ch]
                c0 = chunk_off[ch]
                clo = max(0, c0 - SH)
                chi = min(W, c0 + cw + SH)
                ncols = chi - clo
                ulo = clo - (c0 - SH)
                x3, x4 = x_tiles[it % 3]
                # image-edge pad cols: re-zero on edge chunks (persistent
                # slabs hold stale full-width data from 3 chunks ago)
                if ulo > 0:
                    nc.vector.memset(x3[:, :, 0:ulo], 0.0)
                if ulo + ncols < u:
                    nc.vector.memset(x3[:, :, ulo + ncols:u], 0.0)
                us = slice(ulo, ulo + ncols)
                cs = slice(clo, chi)
                for img in range(IMGS):
                    b = 64 * img
                    eng = nc.scalar if img == 0 else nc.sync
                    # main 16-row blocks: partitions 1..63 <- rows 16q-4..
                    eng.dma_start(out=x3[b + 1:b + 64, 0:RB, us],
                                  in_=xv4[img][:, :, cs])
                    # halo rows 16..23 of partitions 0..62 <- rows 16q+12..
                    eng.dma_start(out=x3[b:b + 63, RB:XR, us],
                                  in_=xv4[img][:, 0:HALO, cs])
                    # q=0: image rows 0..11 land in tile rows 4..15
                    eng.dma_start(out=x3[b:b + 1, SH:RB, us],
                                  in_=xtop[img][:, :, cs])
                    # q=63: image rows 1020..1023 land in halo rows 16..19
                    eng.dma_start(out=x3[b + 63:b + 64, RB:RB + SH, us],
                                  in_=xbot[img][:, :, cs])
                return x3

            def emit_vtree(it, x3):
                ch = it % nchunk
                u = U[ch]

                # first level converts f32 -> fp16 (runs at the f32 rate)
                T2 = pt2.tile([128, 23 * UMAX], f16, tag="t2")
                t2 = T2.rearrange("p (r u) -> p r u", u=UMAX)
                nc.vector.tensor_max(t2[:, 0:23, 0:u], x3[:, 0:23, 0:u], x3[:, 1:24, 0:u])

                T4 = pt4.tile([128, 20 * UMAX], f16, tag="t4")
                t4 = T4.rearrange("p (r u) -> p r u", u=UMAX)
                nc.vector.tensor_max(t4[:, 0:20, 0:u], t2[:, 0:20, 0:u], t2[:, 2:22, 0:u])

                T8 = pt8.tile([128, RB * UMAX], f16, tag="t8")
                t8 = T8.rearrange("p (r u) -> p r u", u=UMAX)
                nc.vector.tensor_max(t8[:, 0:16, 0:u], t4[:, 0:16, 0:u], t4[:, 4:20, 0:u])

                # rows r..r+7 (T8) plus rows r+7..r+8 (T2[r+7]) = rows r..r+8
                R9 = pr9.tile([128, RB * UMAX], f16, tag="r9")
                r9 = R9.rearrange("p (r u) -> p r u", u=UMAX)
                nc.vector.tensor_max(r9[:, 0:16, 0:u], t8[:, 0:16, 0:u], t2[:, 7:23, 0:u])
                return r9

            def emit_htree(it, r9):
                ch = it % nchunk
                cw = CHUNKS[ch]
                u = U[ch]

                G2 = pg.tile([128, RB * UMAX], f16, tag="g")
                g2 = G2.rearrange("p (r u) -> p r u", u=UMAX)
                nc.vector.tensor_max(g2[:, :, 0:u - 2], r9[:, :, 0:u - 2], r9[:, :, 1:u - 1])

                G4 = pg.tile([128, RB * UMAX], f16, tag="g")
                g4 = G4.rearrange("p (r u) -> p r u", u=UMAX)
                nc.vector.tensor_max(g4[:, :, 0:u - 4], g2[:, :, 0:u - 4], g2[:, :, 2:u - 2])

                G8 = pg.tile([128, RB * UMAX], f16, tag="g")
                g8 = G8.rearrange("p (r u) -> p r u", u=UMAX)
                nc.vector.tensor_max(g8[:, :, 0:cw], g4[:, :, 0:cw], g4[:, :, 4:cw + 4])

                OUT = pout.tile([128, RB * UMAX], f32, tag="out")
                o3 = OUT.rearrange("p (r u) -> p r u", u=UMAX)
                o4 = OUT.rearrange("(i p) (r u) -> i p r u", i=IMGS, u=UMAX)
                nc.vector.tensor_max(o3[:, :, 0:cw], g8[:, :, 0:cw], r9[:, :, 8:cw + 8])
                return o3, o4

            def emit_stores(it, o3, o4):
                ch = it % nchunk
                cw = CHUNKS[ch]
                c0 = chunk_off[ch]
                for img in range(IMGS):
                    b = 64 * img
                    eng = nc.sync if img == 0 else nc.scalar
                    eng.dma_start(
                        out=yv4[img][:, :, c0:c0 + cw],
                        in_=o3[b:b + 64, :, 0:cw])

            # --- software-pipelined emission (loads prefetch 2 ahead) ---
            niter = nchunk * reps
            xp = {0: emit_load(0)}
            if niter > 1:
                xp[1] = emit_load(1)
            for it in range(niter):
                if it + 2 < niter:
                    xp[it + 2] = emit_load(it + 2)
                r9 = emit_vtree(it, xp.pop(it))
                o3, o4 = emit_htree(it, r9)
                emit_stores(it, o3, o4)

    nc.compile()
    return nc


def kernel(label):
    lab = np.ascontiguousarray(
        np.asarray(label, dtype=np.float32).reshape(B, H, W)
    )
    if "nc" not in _CACHE:
        _CACHE["nc"] = _build()
    nc = _CACHE["nc"]

    from concourse.bass_utils import run_bass_kernel_spmd

    in_maps = [{"x": lab[IMGS * c:IMGS * (c + 1)]} for c in range(NCORES)]
    res = run_bass_kernel_spmd(nc, in_maps, core_ids=list(range(NCORES)))
    out = np.concatenate([res.results[c]["y"] for c in range(NCORES)], axis=0)
    return out.reshape(B, H, W, 1)


# revision 20
# speedup vs baseline: 1.5238x; 1.5238x over previous
"""9x9 morphological dilation (sliding-window max, SAME padding) on Trainium2.

Input : label (16, 1024, 1024, 1) float32, values in [0, 1).
Output: same shape; out[b,i,j] = max over the 9x9 window centered at (i,j),
        clipped to the image (cv2-style border handling for dilate).

Strategy (per NeuronCore; batch is data-parallel over 8 cores, 2 images/core).
Measured on this part, engine/DMA phases barely overlap, so the design
minimizes TOTAL work and instruction count rather than balancing engines:
a minimal load -> DVE-only compute -> store pipeline (~13 instructions per
chunk), with the f32->fp16 conversion fused into the first tree level.

  - SBUF layout: 128 partitions = 2 images x 64 row-blocks.  Partition q
    holds image rows 16q-4 .. 16q+19 (24 rows: the 16-row block shifted by
    the window radius, plus an 8-row halo loaded directly from HBM), so
    R9[q,r] = max over rows (16q+r-4)..(16q+r+4) = output row 16q+r: output
    rows align exactly with partitions and each image chunk stores with ONE
    contiguous DMA.  Pad rows at the image top/bottom stay zero via a
    one-time memset of the persistent X tiles (inputs are >= 0, so 0 is a
    valid -inf substitute).
  - Vertical 9-max: fully local DVE log tree: T2 (f32 inputs, fp16 output --
    the dtype conversion rides the first pass), T4 (+2 rows), T8 (+4 rows),
    R9[r] = max(T8[r], T2[r+7]) covering rows r..r+8.
  - Horizontal 9-max: DVE log tree along the free dim (+1,+2,+4 col shifts,
    then a merge with R9[c+8] writing f32).  fp16 ops run at the DVE 2x
    rate (~0.5 ns/elem measured); T2 and the merge pay the f32 rate once
    each.
  - Chunks [128, 232, 232, 232, 200]: narrow first chunk shortens the
    startup chain, narrow last shortens the drain.  Loads are prefetched
    two chunks ahead into 3 persistent X tiles and split across the ACT/SP
    DMA queues; stores ride the opposite queue per image.
"""

import numpy as np

B, H, W = 16, 1024, 1024
NCORES = 8
IMGS = 2            # images per core
RB = 16             # rows per partition block
HALO = 8            # vertical halo rows (window 9 -> 8)
SH = 4              # row shift (window radius)
XR = RB + HALO      # rows held per partition (24)
CHUNKS = [128, 232, 232, 232, 200]   # output cols per chunk (sum = 1024)
assert sum(CHUNKS) == W

_CACHE = {}


def _build(reps=1):
    import concourse.bacc as bacc
    import concourse.tile as tile
    import concourse.mybir as mybir

    f32 = mybir.dt.float32
    f16 = mybir.dt.float16

    nc = bacc.Bacc("TRN2", target_bir_lowering=False, debug=False, num_devices=1)
    x = nc.dram_tensor("x", [IMGS, H, W], f32, kind="ExternalInput").ap()
    y = nc.dram_tensor("y", [IMGS, H, W], f32, kind="ExternalOutput").ap()

    # shifted view: entry q (0..62) covers image rows 16q+12 .. 16q+27;
    # used both for the 16-row main blocks (-> partitions q+1) and, via its
    # first 8 rows, for the halo rows of partitions 0..62.  The leading i
    # dim lets one DMA cover both images (partition AP [2, 63]).
    xv4 = x[:, RB - SH:H - SH, :].rearrange("i (q r) c -> i q r c", r=RB)
    xtop = x[:, 0:RB - SH, :].rearrange("i (q r) c -> i q r c", q=1)
    xbot = x[:, H - SH:H, :].rearrange("i (q r) c -> i q r c", q=1)
    yv4 = y.rearrange("i (q r) c -> i q r c", r=RB)

    nchunk = len(CHUNKS)
    chunk_off = np.cumsum([0] + CHUNKS[:-1]).tolist()
    U = [cw + 2 * SH for cw in CHUNKS]
    UMAX = max(U)

    with tile.TileContext(nc) as tc:
        with (
            tc.tile_pool(name="px", bufs=1) as px,
            tc.tile_pool(name="pt2", bufs=1) as pt2,
            tc.tile_pool(name="pt4", bufs=1) as pt4,
            tc.tile_pool(name="pt8", bufs=1) as pt8,
            tc.tile_pool(name="pr9", bufs=2) as pr9,
            tc.tile_pool(name="pg", bufs=2) as pg,
            tc.tile_pool(name="pout", bufs=1) as pout,
        ):
            # three persistent f32 input tiles (prefetch depth 2).  One-time
            # full zeroing keeps the image top/bottom pad rows (q=0 tile
            # rows 0..3, q=63 tile rows 20..23) valid forever; per-chunk
            # loads only overwrite the real-data regions.
            x_tiles = []
            for s in range(3):
                t = px.tile([128, XR * UMAX], f32, tag=f"x{s}")
                t3 = t.rearrange("p (r u) -> p r u", u=UMAX)
                t4 = t.rearrange("(i p) (r u) -> i p r u", i=IMGS, u=UMAX)
                nc.gpsimd.memset(t[:], 0.0)
                x_tiles.append((t3, t4))

            def emit_load(it):
                ch = it % nchunk
                cw = CHUNKS[ch]
                u = U[ch]
                c0 = chunk_off[ch]
                clo = max(0, c0 - SH)
                chi = min(W, c0 + cw + SH)
                ncols = chi - clo
                ulo = clo - (c0 - SH)
                x3, x4 = x_tiles[it % 3]
                # image-edge pad cols: re-zero on edge chunks (persistent
                # slabs hold stale full-width data from 3 chunks ago)
                if ulo > 0:
                    nc.vector.memset(x3[:, :, 0:ulo], 0.0)
                if ulo + ncols < u:
                    nc.vector.memset(x3[:, :, ulo + ncols:u], 0.0)
                us = slice(ulo, ulo + ncols)
                cs = slice(clo, chi)
                for img in range(IMGS):
                    b = 64 * img
                    eng = nc.scalar if img == 0 else nc.sync
                    # main 16-row blocks: partitions 1..63 <- rows 16q-4..
                    eng.dma_start(out=x3[b + 1:b + 64, 0:RB, us],
                                  in_=xv4[img][:, :, cs])
                    # q=0: image rows 0..11 land in tile rows 4..15
                    eng.dma_start(out=x3[b:b + 1, SH:RB, us],
                                  in_=xtop[img][:, :, cs])
                    # q=63: image rows 1020..1023 land in halo rows 16..19
                    eng.dma_start(out=x3[b + 63:b + 64, RB:RB + SH, us],
                                  in_=xbot[img][:, :, cs])
                    # halo rows 16..23 <- partition p+1's rows 0..7, on-chip
                    # (saves 5.8MB/core of HBM re-reads vs loading them)
                    eng.dma_start(out=x3[b:b + 63, RB:XR, us],
                                  in_=x3[b + 1:b + 64, 0:HALO, us])
                return x3

            def emit_vtree(it, x3):
                ch = it % nchunk
                u = U[ch]

                # first level converts f32 -> fp16 (runs at the f32 rate)
                T2 = pt2.tile([128, 23 * UMAX], f16, tag="t2")
                t2 = T2.rearrange("p (r u) -> p r u", u=UMAX)
                nc.vector.tensor_max(t2[:, 0:23, 0:u], x3[:, 0:23, 0:u], x3[:, 1:24, 0:u])

                T4 = pt4.tile([128, 20 * UMAX], f16, tag="t4")
                t4 = T4.rearrange("p (r u) -> p r u", u=UMAX)
                nc.vector.tensor_max(t4[:, 0:20, 0:u], t2[:, 0:20, 0:u], t2[:, 2:22, 0:u])

                T8 = pt8.tile([128, RB * UMAX], f16, tag="t8")
                t8 = T8.rearrange("p (r u) -> p r u", u=UMAX)
                nc.vector.tensor_max(t8[:, 0:16, 0:u], t4[:, 0:16, 0:u], t4[:, 4:20, 0:u])

                # rows r..r+7 (T8) plus rows r+7..r+8 (T2[r+7]) = rows r..r+8
                R9 = pr9.tile([128, RB * UMAX], f16, tag="r9")
                r9 = R9.rearrange("p (r u) -> p r u", u=UMAX)
                nc.vector.tensor_max(r9[:, 0:16, 0:u], t8[:, 0:16, 0:u], t2[:, 7:23, 0:u])
                return r9

            def emit_htree(it, r9):
                ch = it % nchunk
                cw = CHUNKS[ch]
                u = U[ch]

                G2 = pg.tile([128, RB * UMAX], f16, tag="g")
                g2 = G2.rearrange("p (r u) -> p r u", u=UMAX)
                nc.vector.tensor_max(g2[:, :, 0:u - 2], r9[:, :, 0:u - 2], r9[:, :, 1:u - 1])

                G4 = pg.tile([128, RB * UMAX], f16, tag="g")
                g4 = G4.rearrange("p (r u) -> p r u", u=UMAX)
                nc.vector.tensor_max(g4[:, :, 0:u - 4], g2[:, :, 0:u - 4], g2[:, :, 2:u - 2])

                G8 = pg.tile([128, RB * UMAX], f16, tag="g")
                g8 = G8.rearrange("p (r u) -> p r u", u=UMAX)
                nc.vector.tensor_max(g8[:, :, 0:cw], g4[:, :, 0:cw], g4[:, :, 4:cw + 4])

                OUT = pout.tile([128, RB * UMAX], f32, tag="out")
                o3 = OUT.rearrange("p (r u) -> p r u", u=UMAX)
                o4 = OUT.rearrange("(i p) (r u) -> i p r u", i=IMGS, u=UMAX)
                nc.vector.tensor_max(o3[:, :, 0:cw], g8[:, :, 0:cw], r9[:, :, 8:cw + 8])
                return o3, o4

            def emit_stores(it, o3, o4):
                ch = it % nchunk
                cw = CHUNKS[ch]
                c0 = chunk_off[ch]
                for img in range(IMGS):
                    b = 64 * img
                    eng = nc.sync if img == 0 else nc.scalar
                    eng.dma_start(
                        out=yv4[img][:, :, c0:c0 + cw],
                        in_=o3[b:b + 64, :, 0:cw])

            # --- software-pipelined emission (loads prefetch 2 ahead) ---
            niter = nchunk * reps
            xp = {0: emit_load(0)}
            if niter > 1:
                xp[1] = emit_load(1)
            for it in range(niter):
                if it + 2 < niter:
                    xp[it + 2] = emit_load(it + 2)
                r9 = emit_vtree(it, xp.pop(it))
                o3, o4 = emit_htree(it, r9)
                emit_stores(it, o3, o4)

    nc.compile()
    return nc


def kernel(label):
    lab = np.ascontiguousarray(
        np.asarray(label, dtype=np.float32).reshape(B, H, W)
    )
    if "nc" not in _CACHE:
        _CACHE["nc"] = _build()
    nc = _CACHE["nc"]

    from concourse.bass_utils import run_bass_kernel_spmd

    in_maps = [{"x": lab[IMGS * c:IMGS * (c + 1)]} for c in range(NCORES)]
    res = run_bass_kernel_spmd(nc, in_maps, core_ids=list(range(NCORES)))
    out = np.concatenate([res.results[c]["y"] for c in range(NCORES)], axis=0)
    return out.reshape(B, H, W, 1)


# revision 21
# speedup vs baseline: 2.4475x; 1.6062x over previous
"""9x9 morphological dilation (sliding-window max, SAME padding) on Trainium2.

Input : label (16, 1024, 1024, 1) float32, values in [0, 1).
Output: same shape; out[b,i,j] = max over the 9x9 window centered at (i,j),
        clipped to the image (cv2-style border handling for dilate).

Strategy (per NeuronCore; batch is data-parallel over 8 cores, 2 images/core).
Measured on this part, engine/DMA phases barely overlap, so the design
minimizes TOTAL work and instruction count rather than balancing engines:
a minimal load -> DVE-only compute -> store pipeline (~13 instructions per
chunk), with the f32->fp16 conversion fused into the first tree level.

  - SBUF layout: 128 partitions = 2 images x 64 row-blocks.  Partition q
    holds image rows 16q-4 .. 16q+19 (24 rows: the 16-row block shifted by
    the window radius, plus an 8-row halo loaded directly from HBM), so
    R9[q,r] = max over rows (16q+r-4)..(16q+r+4) = output row 16q+r: output
    rows align exactly with partitions and each image chunk stores with ONE
    contiguous DMA.  Pad rows at the image top/bottom stay zero via a
    one-time memset of the persistent X tiles (inputs are >= 0, so 0 is a
    valid -inf substitute).
  - Vertical 9-max: fully local DVE log tree: T2 (f32 inputs, fp16 output --
    the dtype conversion rides the first pass), T4 (+2 rows), T8 (+4 rows),
    R9[r] = max(T8[r], T2[r+7]) covering rows r..r+8.
  - Horizontal 9-max: DVE log tree along the free dim (+1,+2,+4 col shifts,
    then a merge with R9[c+8] writing f32).  fp16 ops run at the DVE 2x
    rate (~0.5 ns/elem measured); T2 and the merge pay the f32 rate once
    each.
  - Chunks [128, 232, 232, 232, 200]: narrow first chunk shortens the
    startup chain, narrow last shortens the drain.  Loads are prefetched
    two chunks ahead into 3 persistent X tiles and split across the ACT/SP
    DMA queues; stores ride the opposite queue per image.
"""

import numpy as np

B, H, W = 16, 1024, 1024
NCORES = 8
IMGS = 2            # images per core
RB = 16             # rows per partition block
HALO = 8            # vertical halo rows (window 9 -> 8)
SH = 4              # row shift (window radius)
XR = RB + HALO      # rows held per partition (24)
CHUNKS = [128, 232, 232, 232, 200]   # output cols per chunk (sum = 1024)
assert sum(CHUNKS) == W

_CACHE = {}


def _build(reps=1):
    import concourse.bacc as bacc
    import concourse.tile as tile
    import concourse.mybir as mybir

    f32 = mybir.dt.float32
    f16 = mybir.dt.float16

    nc = bacc.Bacc("TRN2", target_bir_lowering=False, debug=False, num_devices=1)
    x = nc.dram_tensor("x", [IMGS, H, W], f32, kind="ExternalInput").ap()
    y = nc.dram_tensor("y", [IMGS, H, W], f32, kind="ExternalOutput").ap()

    # shifted view: entry q (0..62) covers image rows 16q+12 .. 16q+27;
    # used both for the 16-row main blocks (-> partitions q+1) and, via its
    # first 8 rows, for the halo rows of partitions 0..62.  The leading i
    # dim lets one DMA cover both images (partition AP [2, 63]).
    xv4 = x[:, RB - SH:H - SH, :].rearrange("i (q r) c -> i q r c", r=RB)
    xtop = x[:, 0:RB - SH, :].rearrange("i (q r) c -> i q r c", q=1)
    xbot = x[:, H - SH:H, :].rearrange("i (q r) c -> i q r c", q=1)
    yv4 = y.rearrange("i (q r) c -> i q r c", r=RB)

    nchunk = len(CHUNKS)
    chunk_off = np.cumsum([0] + CHUNKS[:-1]).tolist()
    U = [cw + 2 * SH for cw in CHUNKS]
    UMAX = max(U)

    with tile.TileContext(nc) as tc:
        with (
            tc.tile_pool(name="px", bufs=1) as px,
            tc.tile_pool(name="pt2", bufs=1) as pt2,
            tc.tile_pool(name="pt4", bufs=1) as pt4,
            tc.tile_pool(name="pt8", bufs=1) as pt8,
            tc.tile_pool(name="pr9", bufs=2) as pr9,
            tc.tile_pool(name="pg", bufs=2) as pg,
            tc.tile_pool(name="pout", bufs=1) as pout,
        ):
            # three persistent f32 input tiles (prefetch depth 2).  One-time
            # full zeroing keeps the image top/bottom pad rows (q=0 tile
            # rows 0..3, q=63 tile rows 20..23) valid forever; per-chunk
            # loads only overwrite the real-data regions.
            x_tiles = []
            for s in range(3):
                t = px.tile([128, XR * UMAX], f32, tag=f"x{s}")
                t3 = t.rearrange("p (r u) -> p r u", u=UMAX)
                t4 = t.rearrange("(i p) (r u) -> i p r u", i=IMGS, u=UMAX)
                nc.gpsimd.memset(t[:], 0.0)
                x_tiles.append((t3, t4))

            def emit_load(it):
                ch = it % nchunk
                cw = CHUNKS[ch]
                u = U[ch]
                c0 = chunk_off[ch]
                clo = max(0, c0 - SH)
                chi = min(W, c0 + cw + SH)
                ncols = chi - clo
                ulo = clo - (c0 - SH)
                x3, x4 = x_tiles[it % 3]
                # image-edge pad cols: re-zero on edge chunks (persistent
                # slabs hold stale full-width data from 3 chunks ago)
                if ulo > 0:
                    nc.vector.memset(x3[:, :, 0:ulo], 0.0)
                if ulo + ncols < u:
                    nc.vector.memset(x3[:, :, ulo + ncols:u], 0.0)
                us = slice(ulo, ulo + ncols)
                cs = slice(clo, chi)
                for img in range(IMGS):
                    b = 64 * img
                    eng = nc.scalar if img == 0 else nc.sync
                    # main 16-row blocks: partitions 1..63 <- rows 16q-4..
                    eng.dma_start(out=x3[b + 1:b + 64, 0:RB, us],
                                  in_=xv4[img][:, :, cs])
                for img in range(IMGS):
                    b = 64 * img
                    # halo rows 16..23 <- partition p+1's rows 0..7, on-chip
                    # (saves 5.8MB/core of HBM re-reads vs loading them).
                    # Rides the otherwise-idle gpsimd SWDGE queue so it never
                    # waits behind loads/stores on the ACT/SP rings.
                    nc.gpsimd.dma_start(out=x3[b:b + 63, RB:XR, us],
                                        in_=x3[b + 1:b + 64, 0:HALO, us])
                for img in range(IMGS):
                    b = 64 * img
                    eng = nc.scalar if img == 0 else nc.sync
                    # q=0: image rows 0..11 land in tile rows 4..15
                    eng.dma_start(out=x3[b:b + 1, SH:RB, us],
                                  in_=xtop[img][:, :, cs])
                    # q=63: image rows 1020..1023 land in halo rows 16..19
                    eng.dma_start(out=x3[b + 63:b + 64, RB:RB + SH, us],
                                  in_=xbot[img][:, :, cs])
                return x3

            def emit_vtree(it, x3):
                ch = it % nchunk
                u = U[ch]

                # first level converts f32 -> fp16 (runs at the f32 rate).
                # Rows 0..14 read only main-load rows, so they overlap the
                # in-flight halo DMA; rows 15..22 wait for it.
                T2 = pt2.tile([128, 23 * UMAX], f16, tag="t2")
                t2 = T2.rearrange("p (r u) -> p r u", u=UMAX)
                nc.vector.tensor_max(t2[:, 0:15, 0:u], x3[:, 0:15, 0:u], x3[:, 1:16, 0:u])
                nc.vector.tensor_max(t2[:, 15:23, 0:u], x3[:, 15:23, 0:u], x3[:, 16:24, 0:u])

                T4 = pt4.tile([128, 20 * UMAX], f16, tag="t4")
                t4 = T4.rearrange("p (r u) -> p r u", u=UMAX)
                nc.vector.tensor_max(t4[:, 0:20, 0:u], t2[:, 0:20, 0:u], t2[:, 2:22, 0:u])

                T8 = pt8.tile([128, RB * UMAX], f16, tag="t8")
                t8 = T8.rearrange("p (r u) -> p r u", u=UMAX)
                nc.vector.tensor_max(t8[:, 0:16, 0:u], t4[:, 0:16, 0:u], t4[:, 4:20, 0:u])

                # rows r..r+7 (T8) plus rows r+7..r+8 (T2[r+7]) = rows r..r+8
                R9 = pr9.tile([128, RB * UMAX], f16, tag="r9")
                r9 = R9.rearrange("p (r u) -> p r u", u=UMAX)
                nc.vector.tensor_max(r9[:, 0:16, 0:u], t8[:, 0:16, 0:u], t2[:, 7:23, 0:u])
                return r9

            def emit_htree(it, r9):
                ch = it % nchunk
                cw = CHUNKS[ch]
                u = U[ch]

                G2 = pg.tile([128, RB * UMAX], f16, tag="g")
                g2 = G2.rearrange("p (r u) -> p r u", u=UMAX)
                nc.vector.tensor_max(g2[:, :, 0:u - 2], r9[:, :, 0:u - 2], r9[:, :, 1:u - 1])

                G4 = pg.tile([128, RB * UMAX], f16, tag="g")
                g4 = G4.rearrange("p (r u) -> p r u", u=UMAX)
                nc.vector.tensor_max(g4[:, :, 0:u - 4], g2[:, :, 0:u - 4], g2[:, :, 2:u - 2])

                G8 = pg.tile([128, RB * UMAX], f16, tag="g")
                g8 = G8.rearrange("p (r u) -> p r u", u=UMAX)
                nc.vector.tensor_max(g8[:, :, 0:cw], g4[:, :, 0:cw], g4[:, :, 4:cw + 4])

                OUT = pout.tile([128, RB * UMAX], f32, tag="out")
                o3 = OUT.rearrange("p (r u) -> p r u", u=UMAX)
                o4 = OUT.rearrange("(i p) (r u) -> i p r u", i=IMGS, u=UMAX)
                nc.vector.tensor_max(o3[:, :, 0:cw], g8[:, :, 0:cw], r9[:, :, 8:cw + 8])
                return o3, o4

            def emit_stores(it, o3, o4):
                ch = it % nchunk
                cw = CHUNKS[ch]
                c0 = chunk_off[ch]
                for img in range(IMGS):
                    b = 64 * img
                    eng = nc.sync if img == 0 else nc.scalar
                    eng.dma_start(
                        out=yv4[img][:, :, c0:c0 + cw],
                        in_=o3[b:b + 64, :, 0:cw])

            # --- software-pipelined emission (loads prefetch 2 ahead) ---
            niter = nchunk * reps
            xp = {0: emit_load(0)}
            if niter > 1:
                xp[1] = emit_load(1)
            for it in range(niter):
                if it + 2 < niter:
                    xp[it + 2] = emit_load(it + 2)
                r9 = emit_vtree(it, xp.pop(it))
                o3, o4 = emit_htree(it, r9)
                emit_stores(it, o3, o4)

    nc.compile()
    return nc


def kernel(label):
    lab = np.ascontiguousarray(
        np.asarray(label, dtype=np.float32).reshape(B, H, W)
    )
    if "nc" not in _CACHE:
        _CACHE["nc"] = _build()
    nc = _CACHE["nc"]

    from concourse.bass_utils import run_bass_kernel_spmd

    in_maps = [{"x": lab[IMGS * c:IMGS * (c + 1)]} for c in range(NCORES)]
    res = run_bass_kernel_spmd(nc, in_maps, core_ids=list(range(NCORES)))
    out = np.concatenate([res.results[c]["y"] for c in range(NCORES)], axis=0)
    return out.reshape(B, H, W, 1)


# revision 23
# speedup vs baseline: 1082.6299x; 442.3446x over previous
"""9x9 morphological dilation (sliding-window max, SAME padding) on Trainium2.

Input : label (16, 1024, 1024, 1) float32, values in [0, 1).
Output: same shape; out[b,i,j] = max over the 9x9 window centered at (i,j),
        clipped to the image (cv2-style border handling for dilate).

Strategy (per NeuronCore; batch is data-parallel over 8 cores, 2 images/core).
Measured on this part, engine/DMA phases barely overlap, so the design
minimizes TOTAL work and instruction count rather than balancing engines:
a minimal load -> DVE-only compute -> store pipeline (~13 instructions per
chunk), with the f32->fp16 conversion fused into the first tree level.

  - SBUF layout: 128 partitions = 2 images x 64 row-blocks.  Partition q
    holds image rows 16q-4 .. 16q+19 (24 rows: the 16-row block shifted by
    the window radius, plus an 8-row halo loaded directly from HBM), so
    R9[q,r] = max over rows (16q+r-4)..(16q+r+4) = output row 16q+r: output
    rows align exactly with partitions and each image chunk stores with ONE
    contiguous DMA.  Pad rows at the image top/bottom stay zero via a
    one-time memset of the persistent X tiles (inputs are >= 0, so 0 is a
    valid -inf substitute).
  - Vertical 9-max: fully local DVE log tree: T2 (f32 inputs, fp16 output --
    the dtype conversion rides the first pass), T4 (+2 rows), T8 (+4 rows),
    R9[r] = max(T8[r], T2[r+7]) covering rows r..r+8.
  - Horizontal 9-max: DVE log tree along the free dim (+1,+2,+4 col shifts,
    then a merge with R9[c+8] writing f32).  fp16 ops run at the DVE 2x
    rate (~0.5 ns/elem measured); T2 and the merge pay the f32 rate once
    each.
  - Chunks [128, 232, 232, 232, 200]: narrow first chunk shortens the
    startup chain, narrow last shortens the drain.  Loads are prefetched
    two chunks ahead into 3 persistent X tiles and split across the ACT/SP
    DMA queues; stores ride the opposite queue per image.
"""

import numpy as np

B, H, W = 16, 1024, 1024
NCORES = 8
IMGS = 2            # images per core
RB = 16             # rows per partition block
HALO = 8            # vertical halo rows (window 9 -> 8)
SH = 4              # row shift (window radius)
XR = RB + HALO      # rows held per partition (24)
CHUNKS = [128, 232, 232, 232, 200]   # output cols per chunk (sum = 1024)
assert sum(CHUNKS) == W

_CACHE = {}


def _build(reps=1):
    import concourse.bacc as bacc
    import concourse.tile as tile
    import concourse.mybir as mybir

    f32 = mybir.dt.float32
    f16 = mybir.dt.float16

    nc = bacc.Bacc("TRN2", target_bir_lowering=False, debug=False, num_devices=1)
    x = nc.dram_tensor("x", [IMGS, H, W], f32, kind="ExternalInput").ap()
    y = nc.dram_tensor("y", [IMGS, H, W], f32, kind="ExternalOutput").ap()

    # shifted view: entry q (0..62) covers image rows 16q+12 .. 16q+27;
    # used both for the 16-row main blocks (-> partitions q+1) and, via its
    # first 8 rows, for the halo rows of partitions 0..62.  The leading i
    # dim lets one DMA cover both images (partition AP [2, 63]).
    xv4 = x[:, RB - SH:H - SH, :].rearrange("i (q r) c -> i q r c", r=RB)
    xtop = x[:, 0:RB - SH, :].rearrange("i (q r) c -> i q r c", q=1)
    xbot = x[:, H - SH:H, :].rearrange("i (q r) c -> i q r c", q=1)
    yv4 = y.rearrange("i (q r) c -> i q r c", r=RB)

    nchunk = len(CHUNKS)
    chunk_off = np.cumsum([0] + CHUNKS[:-1]).tolist()
    U = [cw + 2 * SH for cw in CHUNKS]
    UMAX = max(U)

    with tile.TileContext(nc) as tc:
        with (
            tc.tile_pool(name="px", bufs=1) as px,
            tc.tile_pool(name="pxc", bufs=1) as pxc,
            tc.tile_pool(name="pt2", bufs=1) as pt2,
            tc.tile_pool(name="pt4", bufs=1) as pt4,
            tc.tile_pool(name="pt8", bufs=1) as pt8,
            tc.tile_pool(name="pr9", bufs=2) as pr9,
            tc.tile_pool(name="pg", bufs=2) as pg,
            tc.tile_pool(name="pm", bufs=1) as pm,
            tc.tile_pool(name="pout", bufs=1) as pout,
        ):
            # three persistent f32 input tiles (prefetch depth 2).  One-time
            # full zeroing keeps the image top/bottom pad rows (q=0 tile
            # rows 0..3, q=63 tile rows 20..23) valid forever; per-chunk
            # loads only overwrite the real-data regions.
            x_tiles = []
            for s in range(3):
                t = px.tile([128, XR * UMAX], f32, tag=f"x{s}")
                t3 = t.rearrange("p (r u) -> p r u", u=UMAX)
                t4 = t.rearrange("(i p) (r u) -> i p r u", i=IMGS, u=UMAX)
                nc.gpsimd.memset(t[:], 0.0)
                x_tiles.append((t3, t4))

            def emit_load(it):
                ch = it % nchunk
                cw = CHUNKS[ch]
                u = U[ch]
                c0 = chunk_off[ch]
                clo = max(0, c0 - SH)
                chi = min(W, c0 + cw + SH)
                ncols = chi - clo
                ulo = clo - (c0 - SH)
                x3, x4 = x_tiles[it % 3]
                # image-edge pad cols: re-zero on edge chunks (persistent
                # slabs hold stale full-width data from 3 chunks ago)
                if ulo > 0:
                    nc.vector.memset(x3[:, :, 0:ulo], 0.0)
                if ulo + ncols < u:
                    nc.vector.memset(x3[:, :, ulo + ncols:u], 0.0)
                us = slice(ulo, ulo + ncols)
                cs = slice(clo, chi)
                for img in range(IMGS):
                    b = 64 * img
                    eng = nc.scalar if img == 0 else nc.sync
                    # main 16-row blocks: partitions 1..63 <- rows 16q-4..
                    eng.dma_start(out=x3[b + 1:b + 64, 0:RB, us],
                                  in_=xv4[img][:, :, cs])
                for img in range(IMGS):
                    b = 64 * img
                    # halo rows 16..23 <- partition p+1's rows 0..7, on-chip
                    # (saves 5.8MB/core of HBM re-reads vs loading them).
                    # Rides the otherwise-idle gpsimd SWDGE queue so it never
                    # waits behind loads/stores on the ACT/SP rings.
                    nc.gpsimd.dma_start(out=x3[b:b + 63, RB:XR, us],
                                        in_=x3[b + 1:b + 64, 0:HALO, us])
                for img in range(IMGS):
                    b = 64 * img
                    eng = nc.scalar if img == 0 else nc.sync
                    # q=0: image rows 0..11 land in tile rows 4..15
                    eng.dma_start(out=x3[b:b + 1, SH:RB, us],
                                  in_=xtop[img][:, :, cs])
                    # q=63: image rows 1020..1023 land in halo rows 16..19
                    eng.dma_start(out=x3[b + 63:b + 64, RB:RB + SH, us],
                                  in_=xbot[img][:, :, cs])
                return x3

            def emit_xc(it, x3):
                # ACT pre-converts rows 0..11 to fp16 (rounding commutes
                # with max, so downstream results are bit-identical); lets
                # T2's lower half run at the DVE 2x rate instead of f32.
                u = U[it % nchunk]
                XC = pxc.tile([128, 12 * UMAX], f16, tag="xc")
                xc = XC.rearrange("p (r u) -> p r u", u=UMAX)
                nc.scalar.copy(xc[:, :, 0:u], x3[:, 0:12, 0:u])
                return xc

            def emit_vtree(it, x3, xc):
                ch = it % nchunk
                u = U[ch]

                # first level converts f32 -> fp16.  Rows 0..10 read the
                # ACT-converted fp16 copy (2x rate); rows 11..22 read f32
                # (rows 15..22 additionally wait for the halo DMA).
                T2 = pt2.tile([128, 23 * UMAX], f16, tag="t2")
                t2 = T2.rearrange("p (r u) -> p r u", u=UMAX)
                nc.vector.tensor_max(t2[:, 0:11, 0:u], xc[:, 0:11, 0:u], xc[:, 1:12, 0:u])
                nc.vector.tensor_max(t2[:, 11:15, 0:u], x3[:, 11:15, 0:u], x3[:, 12:16, 0:u])
                nc.vector.tensor_max(t2[:, 15:23, 0:u], x3[:, 15:23, 0:u], x3[:, 16:24, 0:u])

                T4 = pt4.tile([128, 20 * UMAX], f16, tag="t4")
                t4 = T4.rearrange("p (r u) -> p r u", u=UMAX)
                nc.vector.tensor_max(t4[:, 0:20, 0:u], t2[:, 0:20, 0:u], t2[:, 2:22, 0:u])

                T8 = pt8.tile([128, RB * UMAX], f16, tag="t8")
                t8 = T8.rearrange("p (r u) -> p r u", u=UMAX)
                nc.vector.tensor_max(t8[:, 0:16, 0:u], t4[:, 0:16, 0:u], t4[:, 4:20, 0:u])

                # rows r..r+7 (T8) plus rows r+7..r+8 (T2[r+7]) = rows r..r+8
                R9 = pr9.tile([128, RB * UMAX], f16, tag="r9")
                r9 = R9.rearrange("p (r u) -> p r u", u=UMAX)
                nc.vector.tensor_max(r9[:, 0:16, 0:u], t8[:, 0:16, 0:u], t2[:, 7:23, 0:u])
                return r9

            def emit_htree(it, r9):
                ch = it % nchunk
                cw = CHUNKS[ch]
                u = U[ch]

                G2 = pg.tile([128, RB * UMAX], f16, tag="g")
                g2 = G2.rearrange("p (r u) -> p r u", u=UMAX)
                nc.vector.tensor_max(g2[:, :, 0:u - 2], r9[:, :, 0:u - 2], r9[:, :, 1:u - 1])

                G4 = pg.tile([128, RB * UMAX], f16, tag="g")
                g4 = G4.rearrange("p (r u) -> p r u", u=UMAX)
                nc.vector.tensor_max(g4[:, :, 0:u - 4], g2[:, :, 0:u - 4], g2[:, :, 2:u - 2])

                G8 = pg.tile([128, RB * UMAX], f16, tag="g")
                g8 = G8.rearrange("p (r u) -> p r u", u=UMAX)
                nc.vector.tensor_max(g8[:, :, 0:cw], g4[:, :, 0:cw], g4[:, :, 4:cw + 4])

                # merge in fp16 (max of fp16s is exactly representable, so
                # this is bit-identical to merging in f32) -- keeps the DVE
                # at the 2x rate; the idle ACT engine widens to f32 for the
                # store.
                M16 = pm.tile([128, RB * UMAX], f16, tag="m")
                m3 = M16.rearrange("p (r u) -> p r u", u=UMAX)
                nc.vector.tensor_max(m3[:, :, 0:cw], g8[:, :, 0:cw], r9[:, :, 8:cw + 8])
                OUT = pout.tile([128, RB * UMAX], f32, tag="out")
                o3 = OUT.rearrange("p (r u) -> p r u", u=UMAX)
                o4 = OUT.rearrange("(i p) (r u) -> i p r u", i=IMGS, u=UMAX)
                nc.scalar.copy(o3[:, :, 0:cw], m3[:, :, 0:cw])
                return o3, o4

            def emit_stores(it, o3, o4):
                ch = it % nchunk
                cw = CHUNKS[ch]
                c0 = chunk_off[ch]
                for img in range(IMGS):
                    b = 64 * img
                    eng = nc.sync if img == 0 else nc.scalar
                    eng.dma_start(
                        out=yv4[img][:, :, c0:c0 + cw],
                        in_=o3[b:b + 64, :, 0:cw])

            # --- software-pipelined emission (loads prefetch 2 ahead) ---
            niter = nchunk * reps
            xp = {0: emit_load(0)}
            xcp = {0: emit_xc(0, xp[0])}
            if niter > 1:
                xp[1] = emit_load(1)
            for it in range(niter):
                if it + 2 < niter:
                    xp[it + 2] = emit_load(it + 2)
                r9 = emit_vtree(it, xp.pop(it), xcp.pop(it))
                if it + 1 < niter:
                    xcp[it + 1] = emit_xc(it + 1, xp[it + 1])
                o3, o4 = emit_htree(it, r9)
                emit_stores(it, o3, o4)

    nc.compile()
    return nc


def kernel(label):
    lab = np.ascontiguousarray(
        np.asarray(label, dtype=np.float32).reshape(B, H, W)
    )
    if "nc" not in _CACHE:
        _CACHE["nc"] = _build()
    nc = _CACHE["nc"]

    from concourse.bass_utils import run_bass_kernel_spmd

    in_maps = [{"x": lab[IMGS * c:IMGS * (c + 1)]} for c in range(NCORES)]
    res = run_bass_kernel_spmd(nc, in_maps, core_ids=list(range(NCORES)))
    out = np.concatenate([res.results[c]["y"] for c in range(NCORES)], axis=0)
    return out.reshape(B, H, W, 1)
